# revision 1
# baseline (speedup 1.0000x reference)
"""Trainium2 Bass kernel for ContrastiveHessianCalculator GGN-diagonal.

Math (see docstring of the reference):
  out = concat([W1d.flat, b1d, W2d.flat, b2d])   # [164416]
  c_i = sum_o W2[o,i]^2
  For a pair batch (ia, ib):
    h = tanh(x @ W1.T + b1); d = 1 - h^2 (per side a/b)
    W1d[i,j] = c_i * sum_p (da^2 xa_j^2 - 2 da db xa_j xb_j + db^2 xb_j^2)
    b1d[i]   = c_i * sum_p (da - db)^2
    W2d[o,i] = sum_p (ha - hb)^2   (same for every o);  b2d = 0
  out = pos-pairs - neg-pairs.

The p-sum is a matmul:  W1d_raw = U^T @ V  with U k-tiles
[da^2, -2dadb, db^2, hd] (pos and neg) and V k-tiles being the matching
x-products (negated for neg).  b1d comes from an extra all-{+1,-1} column
of V; hd from a one-hot column.  Sharding: data-parallel over the pair
dim P across 8 cores (P/8=128 pairs each -> every tile is exactly one
128-partition tile), AllReduce of the [128,4,258] partial, identical
final assembly on every core.
"""

import numpy as np

import concourse.bass as bass
import concourse.tile as tile
from concourse import bacc, bass_utils, mybir
from concourse.masks import make_identity

F32 = mybir.dt.float32
I32 = mybir.dt.int32
AF = mybir.ActivationFunctionType
ALU = mybir.AluOpType

N, D, H, O, P = 50000, 256, 512, 64, 1024
NCORES = 8
PP = P // NCORES          # 128 pairs per core per pos/neg block
HC = H // 128             # 4 h-chunks
DC = D // 128             # 2 d-chunks
NPARAM = H * D + H + O * H + O  # 164416
VW = D + 2                # V tile width: 256 data + b1-ones col + hd one-hot col

_CACHE = {}


def _build_program():
    nc = bacc.Bacc(
        "TRN2",
        debug=False,
        enable_asserts=False,
        target_bir_lowering=False,
        num_devices=NCORES,
    )

    x_d = nc.dram_tensor("x", [N, D], F32, kind="ExternalInput").ap()
    w1_d = nc.dram_tensor("W1", [H, D], F32, kind="ExternalInput").ap()
    b1_d = nc.dram_tensor("b1r", [1, H], F32, kind="ExternalInput").ap()
    w2_d = nc.dram_tensor("W2", [O, H], F32, kind="ExternalInput").ap()
    idx_d = nc.dram_tensor("idx", [PP, 4], I32, kind="ExternalInput").ap()
    # per-core output: this core's ReduceScatter shard of the summed
    # [128, HC, VW] partial (W1d rows + b1d col + hd col)
    shard_d = nc.dram_tensor(
        "shard", [128 // NCORES, HC, VW], F32, kind="ExternalOutput"
    ).ap()

    with tile.TileContext(nc) as tc:
        _body(tc, x_d, w1_d, b1_d, w2_d, idx_d, shard_d)
    nc.compile()
    return nc


def _body(tc, x_d, w1_d, b1_d, w2_d, idx_d, shard_d):
    nc = tc.nc
    from contextlib import ExitStack

    ctx = ExitStack()
    singles = ctx.enter_context(tc.tile_pool(name="singles", bufs=1))
    work = ctx.enter_context(tc.tile_pool(name="work", bufs=1))
    ps_z = ctx.enter_context(tc.tile_pool(name="ps_z", bufs=2, space="PSUM"))
    ps_t = ctx.enter_context(tc.tile_pool(name="ps_t", bufs=4, space="PSUM"))
    ps_w = ctx.enter_context(tc.tile_pool(name="ps_w", bufs=2, space="PSUM"))
    dram = ctx.enter_context(tc.tile_pool(name="dram", bufs=1, space="DRAM"))

    ident = singles.tile([128, 128], F32)
    make_identity(nc, ident[:])
    ones_r = singles.tile([1, 128], F32)
    nc.vector.memset(ones_r[:], 1.0)
    ones64 = singles.tile([O, 1], F32)
    nc.vector.memset(ones64[:], 1.0)

    # ---- gathers first: idx load gates them, they gate everything ----
    idx_sb = singles.tile([PP, 4], I32)
    nc.sync.dma_start(out=idx_sb[:], in_=idx_d[:])
    xg = [work.tile([128, D], F32, name=f"xg{j}") for j in range(4)]
    for j in range(4):
        nc.gpsimd.indirect_dma_start(
            out=xg[j][:],
            out_offset=None,
            in_=x_d[:],
            in_offset=bass.IndirectOffsetOnAxis(ap=idx_sb[:, j : j + 1], axis=0),
        )

    # ---- weight/bias loads overlap the gathers; W1 split per h-tile ----
    w1_sb = singles.tile([128, HC, D], F32)     # W1 as 4 h-tiles of [128, 256]
    for hc in range(HC):
        nc.sync.dma_start(
            out=w1_sb[:, hc, :], in_=w1_d[hc * 128 : (hc + 1) * 128, :]
        )
    b1row = singles.tile([1, H], F32)
    nc.sync.dma_start(out=b1row[:], in_=b1_d[:])
    w2_sb = singles.tile([O, H], F32)
    nc.sync.dma_start(out=w2_sb[:], in_=w2_d[:])

    # ---- V tiles [128, 258] early: DVE/ACT work independent of matmuls ----
    v_tiles = []
    for blk in range(2):
        sgn = 1.0 if blk == 0 else -1.0
        xa, xb = xg[2 * blk], xg[2 * blk + 1]
        vaa = work.tile([128, VW], F32, name=f"vaa{blk}")
        vab = work.tile([128, VW], F32, name=f"vab{blk}")
        vbb = work.tile([128, VW], F32, name=f"vbb{blk}")
        if blk == 0:
            nc.scalar.square(out=vaa[:, :D], in_=xa[:])
            nc.scalar.square(out=vbb[:, :D], in_=xb[:])
            nc.vector.tensor_mul(vab[:, :D], xa[:], xb[:])
        else:
            nxa = work.tile([128, D], F32, name="nxa")
            nxb = work.tile([128, D], F32, name="nxb")
            nc.vector.tensor_scalar_mul(nxa[:], xa[:], -1.0)
            nc.vector.tensor_scalar_mul(nxb[:], xb[:], -1.0)
            nc.vector.tensor_mul(vaa[:, :D], xa[:], nxa[:])
            nc.vector.tensor_mul(vbb[:, :D], xb[:], nxb[:])
            nc.vector.tensor_mul(vab[:, :D], xa[:], nxb[:])
        for v in (vaa, vab, vbb):
            nc.gpsimd.memset(v[:, D : D + 1], sgn)   # b1d ones column
            nc.gpsimd.memset(v[:, D + 1 : D + 2], 0.0)
        vhd = work.tile([128, VW], F32, name=f"vhd{blk}")
        nc.gpsimd.memset(vhd[:], 0.0)
        nc.gpsimd.memset(vhd[:, D + 1 : D + 2], sgn)  # hd one-hot column
        v_tiles += [vhd, vaa, vab, vbb]

    # ---- all PE transposes batched: W1T chunks then xgT chunks ----
    w1t = [singles.tile([128, H], F32, name=f"w1t{dc}") for dc in range(DC)]
    for dc in range(DC):
        for hc in range(HC):
            tp = ps_t.tile([128, 128], F32, tag="tp")
            nc.tensor.transpose(
                tp[:], w1_sb[:, hc, dc * 128 : (dc + 1) * 128], ident[:]
            )
            nc.vector.tensor_copy(out=w1t[dc][:, hc * 128 : (hc + 1) * 128], in_=tp[:])
    xgt = [[work.tile([128, 128], F32, name=f"xgt{j}_{dc}") for dc in range(DC)]
           for j in range(4)]
    for j in range(4):
        for dc in range(DC):
            tp = ps_t.tile([128, 128], F32, tag="tp")
            nc.tensor.transpose(
                tp[:], xg[j][:, dc * 128 : (dc + 1) * 128], ident[:]
            )
            nc.scalar.copy(out=xgt[j][dc][:], in_=tp[:])

    # ---- b1 broadcast built once; z = xg @ W1.T; tanh(z + b1) ----
    b1p = ps_z.tile([128, H], F32, tag="z", name="b1p")
    nc.tensor.matmul(b1p[:], lhsT=ones_r[:], rhs=b1row[:], start=True, stop=True)
    b1b = singles.tile([128, H], F32)
    nc.scalar.copy(out=b1b[:], in_=b1p[:])
    ha = [work.tile([128, H], F32, name=f"ha{j}") for j in range(4)]
    for j in range(4):
        zp = ps_z.tile([128, H], F32, tag="z")
        for dc in range(DC):
            nc.tensor.matmul(
                zp[:], lhsT=xgt[j][dc][:], rhs=w1t[dc][:],
                start=(dc == 0), stop=(dc == DC - 1),
            )
        zs = work.tile([128, H], F32, name=f"zs{j}")
        nc.vector.tensor_add(zs[:], zp[:], b1b[:])
        nc.scalar.activation(out=ha[j][:], in_=zs[:], func=AF.Tanh)

    # ---- c = colsum(W2^2) as per-partition chunks ----
    w2sq = singles.tile([O, H], F32)
    nc.vector.tensor_mul(w2sq[:], w2_sb[:], w2_sb[:])
    c_sb = singles.tile([128, HC], F32)
    for hc in range(HC):
        cp = ps_t.tile([128, 1], F32, tag="tp", name="cp")
        nc.tensor.matmul(
            cp[:], lhsT=w2sq[:, hc * 128 : (hc + 1) * 128], rhs=ones64[:],
            start=True, stop=True,
        )
        nc.scalar.copy(out=c_sb[:, hc : hc + 1], in_=cp[:])

    # ---- per-block U tiles: [da^2, -2*da*db, db^2, hd]  (sign lives in V) ----
    u_tiles = []   # 8 tiles [128, H], k-order: pos then neg
    for blk in range(2):
        a, b = ha[2 * blk], ha[2 * blk + 1]
        ha_sq = work.tile([128, H], F32, name=f"hasq{blk}")
        hb_sq = work.tile([128, H], F32, name=f"hbsq{blk}")
        nc.vector.tensor_mul(ha_sq[:], a[:], a[:])
        nc.vector.tensor_mul(hb_sq[:], b[:], b[:])
        da_sq = work.tile([128, H], F32, name=f"dasq{blk}")
        db_sq = work.tile([128, H], F32, name=f"dbsq{blk}")
        # (1 - h^2)^2 in one ACT op: Square(-x + 1) applied to h^2
        nc.scalar.activation(out=da_sq[:], in_=ha_sq[:], func=AF.Square,
                             bias=1.0, scale=-1.0)
        nc.scalar.activation(out=db_sq[:], in_=hb_sq[:], func=AF.Square,
                             bias=1.0, scale=-1.0)
        da = work.tile([128, H], F32, name=f"da{blk}")
        db = work.tile([128, H], F32, name=f"db{blk}")
        nc.vector.tensor_scalar(da[:], ha_sq[:], -1.0, 1.0, ALU.mult, ALU.add)
        nc.vector.tensor_scalar(db[:], hb_sq[:], -1.0, 1.0, ALU.mult, ALU.add)
        m2dadb = work.tile([128, H], F32, name=f"m2dadb{blk}")
        nc.vector.tensor_mul(m2dadb[:], da[:], db[:])
        nc.scalar.mul(out=m2dadb[:], in_=m2dadb[:], mul=-2.0)
        hd_d = work.tile([128, H], F32, name=f"hdd{blk}")
        hd = work.tile([128, H], F32, name=f"hd{blk}")
        nc.vector.tensor_sub(hd_d[:], a[:], b[:])
        nc.vector.tensor_mul(hd[:], hd_d[:], hd_d[:])
        u_tiles += [hd, da_sq, m2dadb, db_sq]

    # k-order must pair U with V: pos [daSq,m2dadb,dbSq,hd] x [vaa,vab,vbb,vhd]
    # ---- big matmul + c post-scale -> partial [128, HC, VW] ----
    partial = work.tile([128, HC, VW], F32)
    for hc in range(HC):
        wp = ps_w.tile([128, VW], F32, tag="wp")
        nk = len(u_tiles)
        for k in range(nk):
            nc.tensor.matmul(
                wp[:], lhsT=u_tiles[k][:, hc * 128 : (hc + 1) * 128],
                rhs=v_tiles[k][:], start=(k == 0), stop=(k == nk - 1),
            )
        # rows scale by c (W1d cols 0..255 and the b1d col); hd col copied raw
        if hc % 2 == 0:
            nc.vector.tensor_scalar_mul(
                partial[:, hc, : D + 1], wp[:, : D + 1], c_sb[:, hc : hc + 1]
            )
        else:
            nc.scalar.activation(
                out=partial[:, hc, : D + 1], in_=wp[:, : D + 1],
                func=AF.Copy, scale=c_sb[:, hc : hc + 1],
            )
        nc.vector.tensor_copy(out=partial[:, hc, D + 1 : VW], in_=wp[:, D + 1 : VW])

    # ---- ReduceScatter over the 8 cores: each core keeps a 16-row shard ----
    SH = 128 // NCORES
    cc_in = dram.tile([128, HC, VW], F32)
    rs_out = dram.tile([SH, HC, VW], F32)
    for hc in range(HC):
        nc.sync.dma_start(out=cc_in[:, hc, :], in_=partial[:, hc, :])
    nc.gpsimd.collective_compute(
        "ReduceScatter",
        ALU.add,
        replica_groups=[list(range(NCORES))],
        ins=[cc_in.opt()],
        outs=[rs_out.opt()],
    )
    nc.sync.dma_start(out=shard_d[:], in_=rs_out[:])
    ctx.close()


def _get_program():
    if "nc" not in _CACHE:
        _CACHE["nc"] = _build_program()
    return _CACHE["nc"]


def kernel(**inputs):
    x = np.ascontiguousarray(np.asarray(inputs["x"], dtype=np.float32))
    W1 = np.ascontiguousarray(np.asarray(inputs["W1"], dtype=np.float32))
    b1 = np.ascontiguousarray(
        np.asarray(inputs["b1"], dtype=np.float32).reshape(1, H)
    )
    W2 = np.ascontiguousarray(np.asarray(inputs["W2"], dtype=np.float32))
    iap = np.asarray(inputs["ap"], dtype=np.int32)
    ip = np.asarray(inputs["p"], dtype=np.int32)
    ian = np.asarray(inputs["an"], dtype=np.int32)
    inn = np.asarray(inputs["n"], dtype=np.int32)

    nc = _get_program()
    in_maps = []
    for i in range(NCORES):
        s = slice(i * PP, (i + 1) * PP)
        idx = np.ascontiguousarray(
            np.stack([iap[s], ip[s], ian[s], inn[s]], axis=1).astype(np.int32)
        )
        in_maps.append({"x": x, "W1": W1, "b1r": b1, "W2": W2, "idx": idx})

    res = bass_utils.run_bass_kernel_spmd(
        nc, in_maps, core_ids=list(range(NCORES))
    )
    return _assemble([res.results[c] for c in range(NCORES)])


def _assemble(per_core):
    """Pure gather/unshard: concatenate the ReduceScatter shards and the
    device-computed W2d/b2d tail into the full [164416] output."""
    shards = np.stack([per_core[c]["shard"] for c in range(NCORES)])  # [8,16,HC,VW]
    red = shards.transpose(2, 0, 1, 3).reshape(H, VW)  # h = hc*128 + 16c + q
    out = np.empty(NPARAM, np.float32)
    out[0 : H * D] = red[:, :D].reshape(-1)
    out[H * D : H * D + H] = red[:, D]
    base = H * D + H
    out[base : base + O * H] = np.tile(red[:, D + 1], O)  # W2d rows all equal hd
    out[base + O * H :] = 0.0  # b2d is exactly zero
    return out



# revision 10
# speedup vs baseline: 1.1352x; 1.1352x over previous
"""Trainium2 Bass kernel for ContrastiveHessianCalculator GGN-diagonal.

Math (see the reference):
  out = concat([W1d.flat, b1d, W2d.flat, b2d])   # [164416]
  c_i = sum_o W2[o,i]^2
  For a pair batch (ia, ib):
    h = tanh(x @ W1.T + b1); d = 1 - h^2 (per side a/b)
    W1d[i,j] = c_i * sum_p (da^2 xa_j^2 - 2 da db xa_j xb_j + db^2 xb_j^2)
    b1d[i]   = c_i * sum_p (da - db)^2
    W2d[o,i] = sum_p (ha - hb)^2   (same for every o);  b2d = 0
  out = pos-pairs - neg-pairs.

The p-sum is a matmul:  partial = U^T @ V  with U k-tiles
[hd, da^2, da*db, db^2] (pos and neg) and V k-tiles [one-hot, xa^2,
-2*xa*xb, xb^2] (sign-flipped for neg; the -2 of the cross term lives on
the V side).  b1d comes from a constant column of V ({+1,-2,+1} per
U-type), hd from a one-hot column.  Sharding: data-parallel over the
pair dim P across 8 cores (P/8=128 pairs each -> every tile is exactly
one 128-partition tile), ReduceScatter of the [128,4,258] partial,
identical final assembly on every core.

Speed notes (cost-model driven):
  - z-matmuls/transposes run as float32r (1 cyc/col when out free >= 256
    vs 4 for fp32); U/V tiles are bf16 (1 cyc/col matmul + 2x DVE).
  - b1 is added inside the z PSUM accumulation via a k=1 matmul
    (ones x b1row), so tanh reads PSUM directly.
  - da*db = sqrt(da^2 * db^2) (da,db >= 0), saving two DVE ops/block.
  - elementwise work is spread over DVE/ACT/Pool.
"""

import numpy as np

import concourse.bass as bass
import concourse.tile as tile
from concourse import bacc, bass_utils, mybir
from concourse.masks import make_identity

F32 = mybir.dt.float32
F32R = mybir.dt.float32r
BF16 = mybir.dt.bfloat16
I32 = mybir.dt.int32
AF = mybir.ActivationFunctionType
ALU = mybir.AluOpType

N, D, H, O, P = 50000, 256, 512, 64, 1024
NCORES = 8
PP = P // NCORES          # 128 pairs per core per pos/neg block
HC = H // 128             # 4 h-chunks
DC = D // 128             # 2 d-chunks
NPARAM = H * D + H + O * H + O  # 164416
VW = D + 2                # V tile width: 256 data + b1 const col + hd one-hot col

_CACHE = {}


def _build_program():
    nc = bacc.Bacc(
        "TRN2",
        debug=False,
        enable_asserts=False,
        target_bir_lowering=False,
        num_devices=NCORES,
    )

    x_d = nc.dram_tensor("x", [N, D], F32, kind="ExternalInput").ap()
    w1_d = nc.dram_tensor("W1", [H, D], F32, kind="ExternalInput").ap()
    b1_d = nc.dram_tensor("b1r", [1, H], F32, kind="ExternalInput").ap()
    w2_d = nc.dram_tensor("W2", [O, H], F32, kind="ExternalInput").ap()
    idx_d = nc.dram_tensor("idx", [PP, 4], I32, kind="ExternalInput").ap()
    shard_d = nc.dram_tensor(
        "shard", [128 // NCORES, HC, VW], F32, kind="ExternalOutput"
    ).ap()

    with tile.TileContext(nc) as tc:
        _body(tc, x_d, w1_d, b1_d, w2_d, idx_d, shard_d)
    nc.compile()
    return nc


def _body(tc, x_d, w1_d, b1_d, w2_d, idx_d, shard_d):
    nc = tc.nc
    from contextlib import ExitStack

    ctx = ExitStack()
    singles = ctx.enter_context(tc.tile_pool(name="singles", bufs=1))
    work = ctx.enter_context(tc.tile_pool(name="work", bufs=1))
    ps_z = ctx.enter_context(tc.tile_pool(name="ps_z", bufs=2, space="PSUM"))
    ps_t = ctx.enter_context(tc.tile_pool(name="ps_t", bufs=4, space="PSUM"))
    ps_w = ctx.enter_context(tc.tile_pool(name="ps_w", bufs=2, space="PSUM"))
    dram = ctx.enter_context(tc.tile_pool(name="dram", bufs=1, space="DRAM"))

    ident = singles.tile([128, 128], F32)
    make_identity(nc, ident[:])
    ones_rf = singles.tile([1, 128], F32)
    nc.vector.memset(ones_rf[:], 1.0)
    ones_r = singles.tile([1, 128], F32R)
    nc.vector.tensor_copy(out=ones_r[:], in_=ones_rf[:])
    ones64 = singles.tile([O, 1], F32)
    nc.vector.memset(ones64[:], 1.0)

    # ---- gathers first: idx load gates them, they gate everything ----
    idx_sb = singles.tile([PP, 4], I32)
    nc.sync.dma_start(out=idx_sb[:], in_=idx_d[:])
    xg = [work.tile([128, D], F32, name=f"xg{j}") for j in range(4)]
    for j in range(4):
        nc.gpsimd.indirect_dma_start(
            out=xg[j][:],
            out_offset=None,
            in_=x_d[:],
            in_offset=bass.IndirectOffsetOnAxis(ap=idx_sb[:, j : j + 1], axis=0),
        )

    # ---- weight/bias loads overlap the gathers ----
    w1_sb = singles.tile([128, HC, D], F32)
    for hc in range(HC):
        nc.sync.dma_start(
            out=w1_sb[:, hc, :], in_=w1_d[hc * 128 : (hc + 1) * 128, :]
        )
    b1row = singles.tile([1, H], F32)
    nc.sync.dma_start(out=b1row[:], in_=b1_d[:])
    b1row_r = singles.tile([1, H], F32R)
    nc.vector.tensor_copy(out=b1row_r[:], in_=b1row[:])
    w2_sb = singles.tile([O, H], F32)
    nc.sync.dma_start(out=w2_sb[:], in_=w2_d[:])

    # ---- c = colsum(W2^2): early PE work (also warms the PE p-state) ----
    w2sq = singles.tile([O, H], F32)
    nc.vector.tensor_mul(w2sq[:], w2_sb[:], w2_sb[:])
    c_sb = singles.tile([128, HC], F32)
    for hc in range(HC):
        cp = ps_t.tile([128, 1], F32, tag="tp", name="cp")
        nc.tensor.matmul(
            cp[:], lhsT=w2sq[:, hc * 128 : (hc + 1) * 128], rhs=ones64[:],
            start=True, stop=True,
        )
        nc.scalar.copy(out=c_sb[:, hc : hc + 1], in_=cp[:])

    # ---- W1 transposes ----
    w1t = [singles.tile([128, H], F32R, name=f"w1t{dc}") for dc in range(DC)]
    for dc in range(DC):
        for hc in range(HC):
            tp = ps_t.tile([128, 128], F32, tag="tp")
            nc.tensor.transpose(
                tp[:], w1_sb[:, hc, dc * 128 : (dc + 1) * 128], ident[:]
            )
            nc.vector.tensor_copy(out=w1t[dc][:, hc * 128 : (hc + 1) * 128], in_=tp[:])

    # ---- V tiles [128, 258] bf16: DVE/ACT work independent of matmuls ----
    # V k-order per block: [vhd, vaa, vab, vbb]; -2 of the cross term lives
    # here (data cols and the b1 const col).
    v_tiles = []
    for blk in range(2):
        sgn = 1.0 if blk == 0 else -1.0
        xa, xb = xg[2 * blk], xg[2 * blk + 1]
        vaa = work.tile([128, VW], BF16, name=f"vaa{blk}")
        vab = work.tile([128, VW], BF16, name=f"vab{blk}")
        vbb = work.tile([128, VW], BF16, name=f"vbb{blk}")
        if blk == 0:
            nc.scalar.square(out=vaa[:, :D], in_=xa[:])
            nc.scalar.square(out=vbb[:, :D], in_=xb[:])
            t2 = work.tile([128, D], BF16, name="t2p")
            nc.vector.tensor_mul(t2[:], xa[:], xb[:])
            nc.vector.tensor_scalar_mul(vab[:, :D], t2[:], -2.0)
        else:
            nxa = work.tile([128, D], F32, name="nxa")
            nxb = work.tile([128, D], F32, name="nxb")
            nc.vector.tensor_scalar_mul(nxa[:], xa[:], -1.0)
            nc.vector.tensor_scalar_mul(nxb[:], xb[:], -1.0)
            nc.vector.tensor_mul(vaa[:, :D], xa[:], nxa[:])
            nc.vector.tensor_mul(vbb[:, :D], xb[:], nxb[:])
            t2 = work.tile([128, D], BF16, name="t2n")
            nc.vector.tensor_mul(t2[:], nxa[:], xb[:])   # -xa*xb
            nc.vector.tensor_scalar_mul(vab[:, :D], t2[:], -2.0)  # +2*xa*xb
        # constant columns: col D = b1d weight {+1,-2,+1}*sgn, col D+1 = 0
        nc.gpsimd.memset(vaa[:, D : D + 1], sgn)
        nc.gpsimd.memset(vab[:, D : D + 1], -2.0 * sgn)
        nc.gpsimd.memset(vbb[:, D : D + 1], sgn)
        for v in (vaa, vab, vbb):
            nc.gpsimd.memset(v[:, D + 1 : D + 2], 0.0)
        vhd = work.tile([128, VW], BF16, name=f"vhd{blk}")
        nc.gpsimd.memset(vhd[:], 0.0)
        nc.gpsimd.memset(vhd[:, D + 1 : D + 2], sgn)  # hd one-hot column
        v_tiles += [vhd, vaa, vab, vbb]

    # ---- x transposes + z = [1,x] @ [b1;W1.T] + tanh, per gather j ----
    xgt = [[work.tile([128, 128], F32R, name=f"xgt{j}_{dc}") for dc in range(DC)]
           for j in range(4)]
    ha = [work.tile([128, H], BF16, name=f"ha{j}") for j in range(4)]
    copy_eng = [nc.vector.tensor_copy, nc.scalar.copy]
    for j in range(4):
        for dc in range(DC):
            tp = ps_t.tile([128, 128], F32, tag="tp")
            nc.tensor.transpose(
                tp[:], xg[j][:, dc * 128 : (dc + 1) * 128], ident[:]
            )
            copy_eng[(2 * j + dc) % 2](out=xgt[j][dc][:], in_=tp[:])
        zp = ps_z.tile([128, H], F32, tag="z")
        # b1 rides the accumulation as a k=1 matmul (ones^T x b1row)
        nc.tensor.matmul(
            zp[:], lhsT=ones_r[:], rhs=b1row_r[:], start=True, stop=False,
        )
        for dc in range(DC):
            nc.tensor.matmul(
                zp[:], lhsT=xgt[j][dc][:], rhs=w1t[dc][:],
                start=False, stop=(dc == DC - 1),
            )
        nc.scalar.activation(out=ha[j][:], in_=zp[:], func=AF.Tanh)

    # ---- per-block U tiles bf16: [hd, da^2, da*db, db^2] ----
    u_tiles = []
    for blk in range(2):
        a, b = ha[2 * blk], ha[2 * blk + 1]
        ha_sq = work.tile([128, H], BF16, name=f"hasq{blk}")
        hb_sq = work.tile([128, H], BF16, name=f"hbsq{blk}")
        nc.vector.tensor_mul(ha_sq[:], a[:], a[:])
        nc.vector.tensor_mul(hb_sq[:], b[:], b[:])
        da_sq = work.tile([128, H], BF16, name=f"dasq{blk}")
        db_sq = work.tile([128, H], BF16, name=f"dbsq{blk}")
        # (1 - h^2)^2 in one ACT op: Square(-x + 1) applied to h^2
        nc.scalar.activation(out=da_sq[:], in_=ha_sq[:], func=AF.Square,
                             bias=1.0, scale=-1.0)
        nc.scalar.activation(out=db_sq[:], in_=hb_sq[:], func=AF.Square,
                             bias=1.0, scale=-1.0)
        # da*db = sqrt(da^2 * db^2): da,db >= 0
        dd2 = work.tile([128, H], BF16, name=f"dd2{blk}")
        nc.vector.tensor_mul(dd2[:], da_sq[:], db_sq[:])
        dadb = work.tile([128, H], BF16, name=f"dadb{blk}")
        nc.scalar.activation(out=dadb[:], in_=dd2[:], func=AF.Sqrt)
        hd_d = work.tile([128, H], BF16, name=f"hdd{blk}")
        hd = work.tile([128, H], BF16, name=f"hd{blk}")
        nc.gpsimd.tensor_sub(hd_d[:], a[:], b[:])
        nc.vector.tensor_mul(hd[:], hd_d[:], hd_d[:])
        u_tiles += [hd, da_sq, dadb, db_sq]

    # ---- big matmul (bf16) + c post-scale -> partial [128, HC, VW] ----
    partial = work.tile([128, HC, VW], F32)
    cc_in = dram.tile([128, HC, VW], F32)
    for hc in range(HC):
        wp = ps_w.tile([128, VW], F32, tag="wp")
        nk = len(u_tiles)
        for k in range(nk):
            nc.tensor.matmul(
                wp[:], lhsT=u_tiles[k][:, hc * 128 : (hc + 1) * 128],
                rhs=v_tiles[k][:], start=(k == 0), stop=(k == nk - 1),
            )
        # W1d cols + b1d col scale by c; hd col copied raw (Pool)
        if hc % 2 == 0:
            nc.vector.tensor_scalar_mul(
                partial[:, hc, : D + 1], wp[:, : D + 1], c_sb[:, hc : hc + 1]
            )
        else:
            nc.scalar.activation(
                out=partial[:, hc, : D + 1], in_=wp[:, : D + 1],
                func=AF.Copy, scale=c_sb[:, hc : hc + 1],
            )
        # Pool can't read PSUM; alternate the raw hd-col copy opposite the scale
        if hc % 2 == 0:
            nc.scalar.copy(out=partial[:, hc, D + 1 : VW], in_=wp[:, D + 1 : VW])
        else:
            nc.vector.tensor_copy(out=partial[:, hc, D + 1 : VW], in_=wp[:, D + 1 : VW])
        nc.sync.dma_start(out=cc_in[:, hc, :], in_=partial[:, hc, :])

    # ---- ReduceScatter over the 8 cores: each core keeps a 16-row shard ----
    SH = 128 // NCORES
    rs_out = dram.tile([SH, HC, VW], F32)
    nc.gpsimd.collective_compute(
        "ReduceScatter",
        ALU.add,
        replica_groups=[list(range(NCORES))],
        ins=[cc_in.opt()],
        outs=[rs_out.opt()],
    )
    nc.sync.dma_start(out=shard_d[:], in_=rs_out[:])
    ctx.close()


def _get_program():
    if "nc" not in _CACHE:
        _CACHE["nc"] = _build_program()
    return _CACHE["nc"]


def kernel(**inputs):
    x = np.ascontiguousarray(np.asarray(inputs["x"], dtype=np.float32))
    W1 = np.ascontiguousarray(np.asarray(inputs["W1"], dtype=np.float32))
    b1 = np.ascontiguousarray(
        np.asarray(inputs["b1"], dtype=np.float32).reshape(1, H)
    )
    W2 = np.ascontiguousarray(np.asarray(inputs["W2"], dtype=np.float32))
    iap = np.asarray(inputs["ap"], dtype=np.int32)
    ip = np.asarray(inputs["p"], dtype=np.int32)
    ian = np.asarray(inputs["an"], dtype=np.int32)
    inn = np.asarray(inputs["n"], dtype=np.int32)

    nc = _get_program()
    in_maps = []
    for i in range(NCORES):
        s = slice(i * PP, (i + 1) * PP)
        idx = np.ascontiguousarray(
            np.stack([iap[s], ip[s], ian[s], inn[s]], axis=1).astype(np.int32)
        )
        in_maps.append({"x": x, "W1": W1, "b1r": b1, "W2": W2, "idx": idx})

    res = bass_utils.run_bass_kernel_spmd(
        nc, in_maps, core_ids=list(range(NCORES))
    )
    return _assemble([res.results[c] for c in range(NCORES)])


def _assemble(per_core):
    """Pure gather/unshard: concatenate the ReduceScatter shards and the
    device-computed W2d/b2d tail into the full [164416] output."""
    shards = np.stack([per_core[c]["shard"] for c in range(NCORES)])  # [8,16,HC,VW]
    red = shards.transpose(2, 0, 1, 3).reshape(H, VW)  # h = hc*128 + 16c + q
    out = np.empty(NPARAM, np.float32)
    out[0 : H * D] = red[:, :D].reshape(-1)
    out[H * D : H * D + H] = red[:, D]
    base = H * D + H
    out[base : base + O * H] = np.tile(red[:, D + 1], O)  # W2d rows all equal hd
    out[base + O * H :] = 0.0  # b2d is exactly zero
    return out


# revision 15
# speedup vs baseline: 1.1794x; 1.0389x over previous
"""Trainium2 Bass kernel for ContrastiveHessianCalculator GGN-diagonal.

Math (see the reference):
  out = concat([W1d.flat, b1d, W2d.flat, b2d])   # [164416]
  c_i = sum_o W2[o,i]^2
  For a pair batch (ia, ib):
    h = tanh(x @ W1.T + b1); d = 1 - h^2 (per side a/b)
    W1d[i,j] = c_i * sum_p (da^2 xa_j^2 - 2 da db xa_j xb_j + db^2 xb_j^2)
    b1d[i]   = c_i * sum_p (da - db)^2
    W2d[o,i] = sum_p (ha - hb)^2   (same for every o);  b2d = 0
  out = pos-pairs - neg-pairs.

The p-sum is a matmul:  partial = U^T @ V  with U k-tiles
[hd, da^2, da*db, db^2] (pos and neg) and V k-tiles [one-hot, xa^2,
-2*xa*xb, xb^2] (sign-flipped for neg; the -2 of the cross term lives on
the V side).  b1d comes from a constant column of V ({+1,-2,+1} per
U-type), hd from a one-hot column.  Sharding: data-parallel over the
pair dim P across 8 cores (P/8=128 pairs each -> every tile is exactly
one 128-partition tile), ReduceScatter of the [128,4,258] partial,
identical final assembly on every core.

Speed notes (cost-model driven):
  - z-matmuls/transposes run as float32r (1 cyc/col when out free >= 256
    vs 4 for fp32); U/V tiles are bf16 (1 cyc/col matmul + 2x DVE).
  - b1 is added inside the z PSUM accumulation via a k=1 matmul
    (ones x b1row), so tanh reads PSUM directly.
  - da*db = sqrt(da^2 * db^2) (da,db >= 0), saving two DVE ops/block.
  - elementwise work is spread over DVE/ACT/Pool.
"""

import numpy as np

import concourse.bass as bass
import concourse.tile as tile
from concourse import bacc, bass_utils, mybir
from concourse.masks import make_identity

F32 = mybir.dt.float32
F32R = mybir.dt.float32r
BF16 = mybir.dt.bfloat16
I32 = mybir.dt.int32
AF = mybir.ActivationFunctionType
ALU = mybir.AluOpType

N, D, H, O, P = 50000, 256, 512, 64, 1024
NCORES = 8
PP = P // NCORES          # 128 pairs per core per pos/neg block
HC = H // 128             # 4 h-chunks
DC = D // 128             # 2 d-chunks
NPARAM = H * D + H + O * H + O  # 164416
VW = D + 2                # V tile width: 256 data + b1 const col + hd one-hot col

_CACHE = {}


def _build_program():
    nc = bacc.Bacc(
        "TRN2",
        debug=False,
        enable_asserts=False,
        target_bir_lowering=False,
        num_devices=NCORES,
    )

    x_d = nc.dram_tensor("x", [N, D], F32, kind="ExternalInput").ap()
    w1_d = nc.dram_tensor("W1", [H, D], F32, kind="ExternalInput").ap()
    b1_d = nc.dram_tensor("b1r", [1, H], F32, kind="ExternalInput").ap()
    w2_d = nc.dram_tensor("W2", [O, H], F32, kind="ExternalInput").ap()
    idx_d = nc.dram_tensor("idx", [PP, 4], I32, kind="ExternalInput").ap()
    shard_d = nc.dram_tensor(
        "shard", [128 // NCORES, HC, VW], F32, kind="ExternalOutput"
    ).ap()

    with tile.TileContext(nc) as tc:
        _body(tc, x_d, w1_d, b1_d, w2_d, idx_d, shard_d)
    nc.compile()
    return nc


def _body(tc, x_d, w1_d, b1_d, w2_d, idx_d, shard_d):
    nc = tc.nc
    from contextlib import ExitStack

    ctx = ExitStack()
    singles = ctx.enter_context(tc.tile_pool(name="singles", bufs=1))
    work = ctx.enter_context(tc.tile_pool(name="work", bufs=1))
    ps_z = ctx.enter_context(tc.tile_pool(name="ps_z", bufs=2, space="PSUM"))
    ps_t = ctx.enter_context(tc.tile_pool(name="ps_t", bufs=2, space="PSUM"))
    ps_w = ctx.enter_context(tc.tile_pool(name="ps_w", bufs=4, space="PSUM"))
    dram = ctx.enter_context(tc.tile_pool(name="dram", bufs=1, space="DRAM"))

    ident = singles.tile([128, 128], F32)
    make_identity(nc, ident[:])
    ones_rf = singles.tile([1, 128], F32)
    nc.vector.memset(ones_rf[:], 1.0)
    ones_r = singles.tile([1, 128], F32R)
    nc.vector.tensor_copy(out=ones_r[:], in_=ones_rf[:])
    ones64 = singles.tile([O, 1], F32)
    nc.vector.memset(ones64[:], 1.0)

    # ---- gathers first: idx load gates them, they gate everything ----
    # one batched 2-index gather per pos/neg block (1 SWDGE descgen each)
    idx_sb = singles.tile([PP, 4], I32)
    nc.sync.dma_start(out=idx_sb[:], in_=idx_d[:])
    xgt_tiles = [work.tile([128, D], F32, name=f"xg{j}") for j in range(4)]
    for j in range(4):
        nc.gpsimd.indirect_dma_start(
            out=xgt_tiles[j][:],
            out_offset=None,
            in_=x_d[:],
            in_offset=bass.IndirectOffsetOnAxis(ap=idx_sb[:, j : j + 1], axis=0),
        )
    xg = [t[:] for t in xgt_tiles]

    # ---- weight/bias loads overlap the gathers ----
    w1_sb = singles.tile([128, HC, D], F32)
    for hc in range(HC):
        nc.sync.dma_start(
            out=w1_sb[:, hc, :], in_=w1_d[hc * 128 : (hc + 1) * 128, :]
        )
    b1row = singles.tile([1, H], F32)
    nc.sync.dma_start(out=b1row[:], in_=b1_d[:])
    b1row_r = singles.tile([1, H], F32R)
    nc.vector.tensor_copy(out=b1row_r[:], in_=b1row[:])
    w2_sb = singles.tile([O, H], F32)
    nc.sync.dma_start(out=w2_sb[:], in_=w2_d[:])

    # ---- c = colsum(W2^2): early PE work (also warms the PE p-state) ----
    w2sq = singles.tile([O, H], F32)
    nc.vector.tensor_mul(w2sq[:], w2_sb[:], w2_sb[:])
    c_sb = singles.tile([128, HC], F32)
    for hc in range(HC):
        cp = ps_t.tile([128, 1], F32, tag="tp", name="cp")
        nc.tensor.matmul(
            cp[:], lhsT=w2sq[:, hc * 128 : (hc + 1) * 128], rhs=ones64[:],
            start=True, stop=True,
        )
        nc.scalar.copy(out=c_sb[:, hc : hc + 1], in_=cp[:])

    # ---- W1 transposes ----
    w1t = [singles.tile([128, H], F32R, name=f"w1t{dc}") for dc in range(DC)]
    for dc in range(DC):
        for hc in range(HC):
            tp = ps_t.tile([128, 128], F32, tag="tp")
            nc.tensor.transpose(
                tp[:], w1_sb[:, hc, dc * 128 : (dc + 1) * 128], ident[:]
            )
            nc.vector.tensor_copy(out=w1t[dc][:, hc * 128 : (hc + 1) * 128], in_=tp[:])

    # ---- per-block pipeline: transposes+z+tanh, then V, then U ----
    # emission order matters: Tile runs each engine's queue in order, so the
    # pos block's full chain is emitted before the neg block's.
    xgt = [[work.tile([128, 128], F32R, name=f"xgt{j}_{dc}") for dc in range(DC)]
           for j in range(4)]
    ha = [work.tile([128, H], BF16, name=f"ha{j}") for j in range(4)]
    copy_eng = [nc.vector.tensor_copy, nc.scalar.copy]
    u_tiles = []
    v_tiles = []
    for blk in range(2):
        sgn = 1.0 if blk == 0 else -1.0
        for j in (2 * blk, 2 * blk + 1):
            for dc in range(DC):
                tp = ps_t.tile([128, 128], F32, tag="tp")
                nc.tensor.transpose(
                    tp[:], xg[j][:, dc * 128 : (dc + 1) * 128], ident[:]
                )
                copy_eng[(2 * j + dc) % 2](out=xgt[j][dc][:], in_=tp[:])
            zp = ps_z.tile([128, H], F32, tag="z")
            # b1 rides the accumulation as a k=1 matmul (ones^T x b1row)
            nc.tensor.matmul(
                zp[:], lhsT=ones_r[:], rhs=b1row_r[:], start=True, stop=False,
            )
            for dc in range(DC):
                nc.tensor.matmul(
                    zp[:], lhsT=xgt[j][dc][:], rhs=w1t[dc][:],
                    start=False, stop=(dc == DC - 1),
                )
            nc.scalar.activation(out=ha[j][:], in_=zp[:], func=AF.Tanh)

        # V tiles [128, 258] bf16; k-order [vhd, vaa, vab, vbb]; the -2 of
        # the cross term lives on the V side (data cols + b1 const col).
        xa, xb = xg[2 * blk], xg[2 * blk + 1]
        vaa = work.tile([128, VW], BF16, name=f"vaa{blk}")
        vab = work.tile([128, VW], BF16, name=f"vab{blk}")
        vbb = work.tile([128, VW], BF16, name=f"vbb{blk}")
        if blk == 0:
            nc.scalar.square(out=vaa[:, :D], in_=xa[:])
            nc.scalar.square(out=vbb[:, :D], in_=xb[:])
            t2 = work.tile([128, D], BF16, name="t2p")
            nc.vector.tensor_mul(t2[:], xa[:], xb[:])
            nc.vector.tensor_scalar_mul(vab[:, :D], t2[:], -2.0)
        else:
            nxa = work.tile([128, D], F32, name="nxa")
            nxb = work.tile([128, D], F32, name="nxb")
            nc.vector.tensor_scalar_mul(nxa[:], xa[:], -1.0)
            nc.vector.tensor_scalar_mul(nxb[:], xb[:], -1.0)
            nc.vector.tensor_mul(vaa[:, :D], xa[:], nxa[:])
            nc.vector.tensor_mul(vbb[:, :D], xb[:], nxb[:])
            t2 = work.tile([128, D], BF16, name="t2n")
            nc.vector.tensor_mul(t2[:], nxa[:], xb[:])   # -xa*xb
            nc.vector.tensor_scalar_mul(vab[:, :D], t2[:], -2.0)  # +2*xa*xb
        # constant columns: col D = b1d weight {+1,-2,+1}*sgn, col D+1 = 0
        nc.gpsimd.memset(vaa[:, D : D + 1], sgn)
        nc.gpsimd.memset(vab[:, D : D + 1], -2.0 * sgn)
        nc.gpsimd.memset(vbb[:, D : D + 1], sgn)
        for v in (vaa, vab, vbb):
            nc.gpsimd.memset(v[:, D + 1 : D + 2], 0.0)
        vhd = work.tile([128, VW], BF16, name=f"vhd{blk}")
        nc.gpsimd.memset(vhd[:], 0.0)
        nc.gpsimd.memset(vhd[:, D + 1 : D + 2], sgn)  # hd one-hot column
        v_tiles += [vhd, vaa, vab, vbb]

        # U tiles bf16: [hd, da^2, da*db, db^2] (no Sqrt: ACT table thrash)
        a, b = ha[2 * blk], ha[2 * blk + 1]
        ha_sq = work.tile([128, H], BF16, name=f"hasq{blk}")
        hb_sq = work.tile([128, H], BF16, name=f"hbsq{blk}")
        nc.vector.tensor_mul(ha_sq[:], a[:], a[:])
        nc.vector.tensor_mul(hb_sq[:], b[:], b[:])
        da_sq = work.tile([128, H], BF16, name=f"dasq{blk}")
        db_sq = work.tile([128, H], BF16, name=f"dbsq{blk}")
        # (1 - h^2)^2 in one ACT op: Square(-x + 1) applied to h^2
        nc.scalar.activation(out=da_sq[:], in_=ha_sq[:], func=AF.Square,
                             bias=1.0, scale=-1.0)
        nc.scalar.activation(out=db_sq[:], in_=hb_sq[:], func=AF.Square,
                             bias=1.0, scale=-1.0)
        da = work.tile([128, H], BF16, name=f"da{blk}")
        db = work.tile([128, H], BF16, name=f"db{blk}")
        nc.vector.tensor_scalar(da[:], ha_sq[:], -1.0, 1.0, ALU.mult, ALU.add)
        nc.vector.tensor_scalar(db[:], hb_sq[:], -1.0, 1.0, ALU.mult, ALU.add)
        dadb = work.tile([128, H], BF16, name=f"dadb{blk}")
        nc.vector.tensor_mul(dadb[:], da[:], db[:])
        hd_d = work.tile([128, H], BF16, name=f"hdd{blk}")
        hd = work.tile([128, H], BF16, name=f"hd{blk}")
        nc.gpsimd.tensor_sub(hd_d[:], a[:], b[:])
        nc.vector.tensor_mul(hd[:], hd_d[:], hd_d[:])
        u_tiles += [hd, da_sq, dadb, db_sq]

    # ---- big matmul (bf16) + c post-scale -> partial [128, HC, VW] ----
    # pos half emitted (and runs) while the neg block is still in prep
    partial = work.tile([128, HC, VW], F32)
    cc_in = dram.tile([128, HC, VW], F32)
    wps = [ps_w.tile([128, VW], F32, tag="wp", name=f"wp{hc}") for hc in range(HC)]
    for half in range(2):
        for hc in range(HC):
            wp = wps[hc]
            for k in range(4):
                kk = 4 * half + k
                nc.tensor.matmul(
                    wp[:], lhsT=u_tiles[kk][:, hc * 128 : (hc + 1) * 128],
                    rhs=v_tiles[kk][:], start=(kk == 0), stop=(kk == 7),
                )
            if half == 0:
                continue
            # W1d cols + b1d col scale by c; hd col copied raw
            if hc % 2 == 0:
                nc.vector.tensor_scalar_mul(
                    partial[:, hc, : D + 1], wp[:, : D + 1], c_sb[:, hc : hc + 1]
                )
                nc.scalar.copy(out=partial[:, hc, D + 1 : VW], in_=wp[:, D + 1 : VW])
            else:
                nc.scalar.activation(
                    out=partial[:, hc, : D + 1], in_=wp[:, : D + 1],
                    func=AF.Copy, scale=c_sb[:, hc : hc + 1],
                )
                nc.vector.tensor_copy(out=partial[:, hc, D + 1 : VW], in_=wp[:, D + 1 : VW])
            nc.sync.dma_start(out=cc_in[:, hc, :], in_=partial[:, hc, :])

    # ---- ReduceScatter over the 8 cores: each core keeps a 16-row shard ----
    SH = 128 // NCORES
    rs_out = dram.tile([SH, HC, VW], F32)
    nc.gpsimd.collective_compute(
        "ReduceScatter",
        ALU.add,
        replica_groups=[list(range(NCORES))],
        ins=[cc_in.opt()],
        outs=[rs_out.opt()],
    )
    nc.sync.dma_start(out=shard_d[:], in_=rs_out[:])
    ctx.close()


def _get_program():
    if "nc" not in _CACHE:
        _CACHE["nc"] = _build_program()
    return _CACHE["nc"]


def kernel(**inputs):
    x = np.ascontiguousarray(np.asarray(inputs["x"], dtype=np.float32))
    W1 = np.ascontiguousarray(np.asarray(inputs["W1"], dtype=np.float32))
    b1 = np.ascontiguousarray(
        np.asarray(inputs["b1"], dtype=np.float32).reshape(1, H)
    )
    W2 = np.ascontiguousarray(np.asarray(inputs["W2"], dtype=np.float32))
    iap = np.asarray(inputs["ap"], dtype=np.int32)
    ip = np.asarray(inputs["p"], dtype=np.int32)
    ian = np.asarray(inputs["an"], dtype=np.int32)
    inn = np.asarray(inputs["n"], dtype=np.int32)

    nc = _get_program()
    in_maps = []
    for i in range(NCORES):
        s = slice(i * PP, (i + 1) * PP)
        idx = np.ascontiguousarray(
            np.stack([iap[s], ip[s], ian[s], inn[s]], axis=1).astype(np.int32)
        )
        in_maps.append({"x": x, "W1": W1, "b1r": b1, "W2": W2, "idx": idx})

    res = bass_utils.run_bass_kernel_spmd(
        nc, in_maps, core_ids=list(range(NCORES))
    )
    return _assemble([res.results[c] for c in range(NCORES)])


def _assemble(per_core):
    """Pure gather/unshard: concatenate the ReduceScatter shards and the
    device-computed W2d/b2d tail into the full [164416] output."""
    shards = np.stack([per_core[c]["shard"] for c in range(NCORES)])  # [8,16,HC,VW]
    red = shards.transpose(2, 0, 1, 3).reshape(H, VW)  # h = hc*128 + 16c + q
    out = np.empty(NPARAM, np.float32)
    out[0 : H * D] = red[:, :D].reshape(-1)
    out[H * D : H * D + H] = red[:, D]
    base = H * D + H
    out[base : base + O * H] = np.tile(red[:, D + 1], O)  # W2d rows all equal hd
    out[base + O * H :] = 0.0  # b2d is exactly zero
    return out


# revision 17
# speedup vs baseline: 1.2394x; 1.0508x over previous
"""Trainium2 Bass kernel for ContrastiveHessianCalculator GGN-diagonal.

Math (see the reference):
  out = concat([W1d.flat, b1d, W2d.flat, b2d])   # [164416]
  c_i = sum_o W2[o,i]^2
  For a pair batch (ia, ib):
    h = tanh(x @ W1.T + b1); d = 1 - h^2 (per side a/b)
    W1d[i,j] = c_i * sum_p (da^2 xa_j^2 - 2 da db xa_j xb_j + db^2 xb_j^2)
    b1d[i]   = c_i * sum_p (da - db)^2
    W2d[o,i] = sum_p (ha - hb)^2   (same for every o);  b2d = 0
  out = pos-pairs - neg-pairs.

The p-sum is a matmul:  partial = U^T @ V  with U k-tiles
[hd, da^2, da*db, db^2] (pos and neg) and V k-tiles [one-hot, xa^2,
-2*xa*xb, xb^2] (sign-flipped for neg; the -2 of the cross term lives on
the V side).  b1d comes from a constant column of V ({+1,-2,+1} per
U-type), hd from a one-hot column.  Sharding: data-parallel over the
pair dim P across 8 cores (P/8=128 pairs each -> every tile is exactly
one 128-partition tile), ReduceScatter of the [128,4,258] partial,
identical final assembly on every core.

Speed notes (cost-model driven):
  - z-matmuls/transposes run as float32r (1 cyc/col when out free >= 256
    vs 4 for fp32); U/V tiles are bf16 (1 cyc/col matmul + 2x DVE).
  - b1 is added inside the z PSUM accumulation via a k=1 matmul
    (ones x b1row), so tanh reads PSUM directly.
  - da*db = sqrt(da^2 * db^2) (da,db >= 0), saving two DVE ops/block.
  - elementwise work is spread over DVE/ACT/Pool.
"""

import numpy as np

import concourse.bass as bass
import concourse.tile as tile
from concourse import bacc, bass_utils, mybir
from concourse.masks import make_identity

F32 = mybir.dt.float32
F32R = mybir.dt.float32r
BF16 = mybir.dt.bfloat16
I32 = mybir.dt.int32
AF = mybir.ActivationFunctionType
ALU = mybir.AluOpType

N, D, H, O, P = 50000, 256, 512, 64, 1024
NCORES = 8
PP = P // NCORES          # 128 pairs per core per pos/neg block
HC = H // 128             # 4 h-chunks
DC = D // 128             # 2 d-chunks
NPARAM = H * D + H + O * H + O  # 164416
VW = D + 2                # V tile width: 256 data + b1 const col + hd one-hot col

_CACHE = {}


def _build_program():
    nc = bacc.Bacc(
        "TRN2",
        debug=False,
        enable_asserts=False,
        target_bir_lowering=False,
        num_devices=NCORES,
    )

    x_d = nc.dram_tensor("x", [N, D], F32, kind="ExternalInput").ap()
    w1_d = nc.dram_tensor("W1", [H, D], F32, kind="ExternalInput").ap()
    b1_d = nc.dram_tensor("b1r", [1, H], F32, kind="ExternalInput").ap()
    w2_d = nc.dram_tensor("W2", [O, H], F32, kind="ExternalInput").ap()
    idx_d = nc.dram_tensor("idx", [PP, 4], I32, kind="ExternalInput").ap()
    shard_d = nc.dram_tensor(
        "shard", [128 // NCORES, HC, VW], F32, kind="ExternalOutput"
    ).ap()

    with tile.TileContext(nc) as tc:
        _body(tc, x_d, w1_d, b1_d, w2_d, idx_d, shard_d)
    nc.compile()
    return nc


def _body(tc, x_d, w1_d, b1_d, w2_d, idx_d, shard_d):
    nc = tc.nc
    from contextlib import ExitStack

    ctx = ExitStack()
    singles = ctx.enter_context(tc.tile_pool(name="singles", bufs=1))
    work = ctx.enter_context(tc.tile_pool(name="work", bufs=1))
    ps_z = ctx.enter_context(tc.tile_pool(name="ps_z", bufs=2, space="PSUM"))
    ps_t = ctx.enter_context(tc.tile_pool(name="ps_t", bufs=2, space="PSUM"))
    ps_w = ctx.enter_context(tc.tile_pool(name="ps_w", bufs=4, space="PSUM"))
    dram = ctx.enter_context(tc.tile_pool(name="dram", bufs=1, space="DRAM"))

    # ---- gathers first: idx load gates them, they gate everything ----
    idx_sb = singles.tile([PP, 4], I32)
    nc.sync.dma_start(out=idx_sb[:], in_=idx_d[:])
    xg_t = [work.tile([128, D], F32, name=f"xg{j}") for j in range(4)]
    for j in range(4):
        nc.gpsimd.indirect_dma_start(
            out=xg_t[j][:],
            out_offset=None,
            in_=x_d[:],
            in_offset=bass.IndirectOffsetOnAxis(ap=idx_sb[:, j : j + 1], axis=0),
        )
    xg = [t[:] for t in xg_t]

    # ---- weight/bias loads overlap the gathers ----
    w1_sb = singles.tile([128, HC, D], F32)
    for hc in range(HC):
        nc.sync.dma_start(
            out=w1_sb[:, hc, :], in_=w1_d[hc * 128 : (hc + 1) * 128, :]
        )
    b1row = singles.tile([1, H], F32)
    nc.sync.dma_start(out=b1row[:], in_=b1_d[:])
    b1row_r = singles.tile([1, H], F32R)
    nc.vector.tensor_copy(out=b1row_r[:], in_=b1row[:])
    w2_sb = singles.tile([O, H], F32)
    nc.sync.dma_start(out=w2_sb[:], in_=w2_d[:])

    ident = singles.tile([128, 128], F32)
    make_identity(nc, ident[:])
    ones_rf = singles.tile([1, 128], F32)
    nc.vector.memset(ones_rf[:], 1.0)
    ones_r = singles.tile([1, 128], F32R)
    nc.vector.tensor_copy(out=ones_r[:], in_=ones_rf[:])
    ones64 = singles.tile([O, 1], F32)
    nc.vector.memset(ones64[:], 1.0)

    # ---- c = colsum(W2^2): early PE work (also warms the PE p-state) ----
    w2sq = singles.tile([O, H], F32)
    nc.vector.tensor_mul(w2sq[:], w2_sb[:], w2_sb[:])
    c_sb = singles.tile([128, HC], F32)
    for hc in range(HC):
        cp = ps_t.tile([128, 1], F32, tag="tp", name="cp")
        nc.tensor.matmul(
            cp[:], lhsT=w2sq[:, hc * 128 : (hc + 1) * 128], rhs=ones64[:],
            start=True, stop=True,
        )
        nc.scalar.copy(out=c_sb[:, hc : hc + 1], in_=cp[:])

    # ---- W1 transposes: 4 per PSUM stage tile, one wide copy each ----
    w1t = [singles.tile([128, H], F32R, name=f"w1t{dc}") for dc in range(DC)]
    for dc in range(DC):
        st = ps_t.tile([128, H], F32, tag="tp", name=f"stw{dc}")
        for hc in range(HC):
            nc.tensor.transpose(
                st[:, hc * 128 : (hc + 1) * 128],
                w1_sb[:, hc, dc * 128 : (dc + 1) * 128], ident[:]
            )
        nc.scalar.copy(out=w1t[dc][:], in_=st[:])

    # ---- per-j: x transpose pair -> one copy -> z (b1 via k=1 matmul) -> tanh
    xgt = [work.tile([128, DC, 128], F32R, name=f"xgt{j}") for j in range(4)]
    ha = [work.tile([128, H], BF16, name=f"ha{j}") for j in range(4)]
    copy_eng = [nc.vector.tensor_copy, nc.scalar.copy]
    for j in range(4):
        st = ps_t.tile([128, DC * 128], F32, tag="tp", name=f"stx{j}")
        for dc in range(DC):
            nc.tensor.transpose(
                st[:, dc * 128 : (dc + 1) * 128],
                xg[j][:, dc * 128 : (dc + 1) * 128], ident[:]
            )
        copy_eng[j % 2](out=xgt[j][:, :, :].opt(), in_=st[:])
        zp = ps_z.tile([128, H], F32, tag="z")
        nc.tensor.matmul(
            zp[:], lhsT=ones_r[:], rhs=b1row_r[:], start=True, stop=False,
        )
        for dc in range(DC):
            nc.tensor.matmul(
                zp[:], lhsT=xgt[j][:, dc, :], rhs=w1t[dc][:],
                start=False, stop=(dc == DC - 1),
            )
        nc.scalar.activation(out=ha[j][:], in_=zp[:], func=AF.Tanh)

    # ---- per-block V and U tiles (bf16) ----
    # k-order per block: U [da^2, db^2, dadb, hd] <-> V [vaa, vbb, vab, vhd]
    # (latest-ready tiles last); the -2 of the cross term lives on V.
    u_tiles = []
    v_tiles = []
    for blk in range(2):
        sgn = 1.0 if blk == 0 else -1.0
        xa, xb = xg[2 * blk], xg[2 * blk + 1]
        vaa = work.tile([128, VW], BF16, name=f"vaa{blk}")
        vab = work.tile([128, VW], BF16, name=f"vab{blk}")
        vbb = work.tile([128, VW], BF16, name=f"vbb{blk}")
        if blk == 0:
            nc.scalar.square(out=vaa[:, :D], in_=xa[:])
            nc.scalar.square(out=vbb[:, :D], in_=xb[:])
            t2 = work.tile([128, D], BF16, name="t2p")
            nc.vector.tensor_mul(t2[:], xa[:], xb[:])
            nc.vector.tensor_scalar_mul(vab[:, :D], t2[:], -2.0)
        else:
            nxa = work.tile([128, D], F32, name="nxa")
            nxb = work.tile([128, D], F32, name="nxb")
            nc.vector.tensor_scalar_mul(nxa[:], xa[:], -1.0)
            nc.vector.tensor_scalar_mul(nxb[:], xb[:], -1.0)
            nc.vector.tensor_mul(vaa[:, :D], xa[:], nxa[:])
            nc.vector.tensor_mul(vbb[:, :D], xb[:], nxb[:])
            t2 = work.tile([128, D], BF16, name="t2n")
            nc.vector.tensor_mul(t2[:], nxa[:], xb[:])   # -xa*xb
            nc.vector.tensor_scalar_mul(vab[:, :D], t2[:], -2.0)  # +2*xa*xb
        # constant columns (DVE memsets keep Pool clear for gather descgen):
        # col D = b1d weight {+1,-2,+1}*sgn, col D+1 = 0 (hd one-hot on vhd)
        nc.vector.memset(vaa[:, D : D + 1], sgn)
        nc.vector.memset(vab[:, D : D + 1], -2.0 * sgn)
        nc.vector.memset(vbb[:, D : D + 1], sgn)
        for v in (vaa, vab, vbb):
            nc.vector.memset(v[:, D + 1 : D + 2], 0.0)
        vhd = work.tile([128, VW], BF16, name=f"vhd{blk}")
        nc.vector.memset(vhd[:], 0.0)
        nc.vector.memset(vhd[:, D + 1 : D + 2], sgn)
        v_tiles += [vaa, vbb, vab, vhd]

        a, b = ha[2 * blk], ha[2 * blk + 1]
        ha_sq = work.tile([128, H], BF16, name=f"hasq{blk}")
        hb_sq = work.tile([128, H], BF16, name=f"hbsq{blk}")
        nc.vector.tensor_mul(ha_sq[:], a[:], a[:])
        nc.vector.tensor_mul(hb_sq[:], b[:], b[:])
        da_sq = work.tile([128, H], BF16, name=f"dasq{blk}")
        db_sq = work.tile([128, H], BF16, name=f"dbsq{blk}")
        # (1 - h^2)^2 in one ACT op: Square(-x + 1) applied to h^2
        nc.scalar.activation(out=da_sq[:], in_=ha_sq[:], func=AF.Square,
                             bias=1.0, scale=-1.0)
        nc.scalar.activation(out=db_sq[:], in_=hb_sq[:], func=AF.Square,
                             bias=1.0, scale=-1.0)
        da = work.tile([128, H], BF16, name=f"da{blk}")
        db = work.tile([128, H], BF16, name=f"db{blk}")
        nc.vector.tensor_scalar(da[:], ha_sq[:], -1.0, 1.0, ALU.mult, ALU.add)
        nc.vector.tensor_scalar(db[:], hb_sq[:], -1.0, 1.0, ALU.mult, ALU.add)
        dadb = work.tile([128, H], BF16, name=f"dadb{blk}")
        nc.vector.tensor_mul(dadb[:], da[:], db[:])
        hd_d = work.tile([128, H], BF16, name=f"hdd{blk}")
        hd = work.tile([128, H], BF16, name=f"hd{blk}")
        nc.gpsimd.tensor_sub(hd_d[:], a[:], b[:])
        nc.scalar.square(out=hd[:], in_=hd_d[:])
        u_tiles += [da_sq, db_sq, dadb, hd]

    # ---- big matmul (bf16) + c post-scale -> partial [128, HC, VW] ----
    # pos half runs while the neg block is still in prep
    partial = work.tile([128, HC, VW], F32)
    cc_in = dram.tile([128, HC, VW], F32)
    wps = [ps_w.tile([128, VW], F32, tag="wp", name=f"wp{hc}") for hc in range(HC)]
    for half in range(2):
        for hc in range(HC):
            wp = wps[hc]
            for k in range(4):
                kk = 4 * half + k
                nc.tensor.matmul(
                    wp[:], lhsT=u_tiles[kk][:, hc * 128 : (hc + 1) * 128],
                    rhs=v_tiles[kk][:], start=(kk == 0), stop=(kk == 7),
                )
            if half == 0:
                continue
            # W1d cols + b1d col scale by c; hd col copied raw
            if hc % 2 == 0:
                nc.vector.tensor_scalar_mul(
                    partial[:, hc, : D + 1], wp[:, : D + 1], c_sb[:, hc : hc + 1]
                )
                nc.scalar.copy(out=partial[:, hc, D + 1 : VW], in_=wp[:, D + 1 : VW])
            else:
                nc.scalar.activation(
                    out=partial[:, hc, : D + 1], in_=wp[:, : D + 1],
                    func=AF.Copy, scale=c_sb[:, hc : hc + 1],
                )
                nc.vector.tensor_copy(out=partial[:, hc, D + 1 : VW], in_=wp[:, D + 1 : VW])
            nc.sync.dma_start(out=cc_in[:, hc, :], in_=partial[:, hc, :])

    # ---- ReduceScatter over the 8 cores (collectives can't write IO) ----
    SH = 128 // NCORES
    rs_out = dram.tile([SH, HC, VW], F32)
    nc.gpsimd.collective_compute(
        "ReduceScatter",
        ALU.add,
        replica_groups=[list(range(NCORES))],
        ins=[cc_in.opt()],
        outs=[rs_out.opt()],
    )
    nc.sync.dma_start(out=shard_d[:], in_=rs_out[:])
    ctx.close()


def _get_program():
    if "nc" not in _CACHE:
        _CACHE["nc"] = _build_program()
    return _CACHE["nc"]


def kernel(**inputs):
    x = np.ascontiguousarray(np.asarray(inputs["x"], dtype=np.float32))
    W1 = np.ascontiguousarray(np.asarray(inputs["W1"], dtype=np.float32))
    b1 = np.ascontiguousarray(
        np.asarray(inputs["b1"], dtype=np.float32).reshape(1, H)
    )
    W2 = np.ascontiguousarray(np.asarray(inputs["W2"], dtype=np.float32))
    iap = np.asarray(inputs["ap"], dtype=np.int32)
    ip = np.asarray(inputs["p"], dtype=np.int32)
    ian = np.asarray(inputs["an"], dtype=np.int32)
    inn = np.asarray(inputs["n"], dtype=np.int32)

    nc = _get_program()
    in_maps = []
    for i in range(NCORES):
        s = slice(i * PP, (i + 1) * PP)
        idx = np.ascontiguousarray(
            np.stack([iap[s], ip[s], ian[s], inn[s]], axis=1).astype(np.int32)
        )
        in_maps.append({"x": x, "W1": W1, "b1r": b1, "W2": W2, "idx": idx})

    res = bass_utils.run_bass_kernel_spmd(
        nc, in_maps, core_ids=list(range(NCORES))
    )
    return _assemble([res.results[c] for c in range(NCORES)])


def _assemble(per_core):
    """Pure gather/unshard: concatenate the ReduceScatter shards and the
    device-computed W2d/b2d tail into the full [164416] output."""
    shards = np.stack([per_core[c]["shard"] for c in range(NCORES)])  # [8,16,HC,VW]
    red = shards.transpose(2, 0, 1, 3).reshape(H, VW)  # h = hc*128 + 16c + q
    out = np.empty(NPARAM, np.float32)
    out[0 : H * D] = red[:, :D].reshape(-1)
    out[H * D : H * D + H] = red[:, D]
    base = H * D + H
    out[base : base + O * H] = np.tile(red[:, D + 1], O)  # W2d rows all equal hd
    out[base + O * H :] = 0.0  # b2d is exactly zero
    return out


# revision 18
# speedup vs baseline: 1.2786x; 1.0317x over previous
"""Trainium2 Bass kernel for ContrastiveHessianCalculator GGN-diagonal.

Math (see the reference):
  out = concat([W1d.flat, b1d, W2d.flat, b2d])   # [164416]
  c_i = sum_o W2[o,i]^2
  For a pair batch (ia, ib):
    h = tanh(x @ W1.T + b1); d = 1 - h^2 (per side a/b)
    W1d[i,j] = c_i * sum_p (da^2 xa_j^2 - 2 da db xa_j xb_j + db^2 xb_j^2)
    b1d[i]   = c_i * sum_p (da - db)^2
    W2d[o,i] = sum_p (ha - hb)^2   (same for every o);  b2d = 0
  out = pos-pairs - neg-pairs.

The p-sum is a matmul:  partial = U^T @ V  with U k-tiles
[hd, da^2, da*db, db^2] (pos and neg) and V k-tiles [one-hot, xa^2,
-2*xa*xb, xb^2] (sign-flipped for neg; the -2 of the cross term lives on
the V side).  b1d comes from a constant column of V ({+1,-2,+1} per
U-type), hd from a one-hot column.  Sharding: data-parallel over the
pair dim P across 8 cores (P/8=128 pairs each -> every tile is exactly
one 128-partition tile), ReduceScatter of the [128,4,258] partial,
identical final assembly on every core.

Speed notes (cost-model driven):
  - z-matmuls/transposes run as float32r (1 cyc/col when out free >= 256
    vs 4 for fp32); U/V tiles are bf16 (1 cyc/col matmul + 2x DVE).
  - b1 is added inside the z PSUM accumulation via a k=1 matmul
    (ones x b1row), so tanh reads PSUM directly.
  - da*db = sqrt(da^2 * db^2) (da,db >= 0), saving two DVE ops/block.
  - elementwise work is spread over DVE/ACT/Pool.
"""

import numpy as np

import concourse.bass as bass
import concourse.tile as tile
from concourse import bacc, bass_utils, mybir
from concourse.masks import make_identity

F32 = mybir.dt.float32
F32R = mybir.dt.float32r
BF16 = mybir.dt.bfloat16
I32 = mybir.dt.int32
AF = mybir.ActivationFunctionType
ALU = mybir.AluOpType

N, D, H, O, P = 50000, 256, 512, 64, 1024
NCORES = 8
PP = P // NCORES          # 128 pairs per core per pos/neg block
HC = H // 128             # 4 h-chunks
DC = D // 128             # 2 d-chunks
NPARAM = H * D + H + O * H + O  # 164416
VW = D + 2                # V tile width: 256 data + b1 const col + hd one-hot col

_CACHE = {}


def _build_program():
    nc = bacc.Bacc(
        "TRN2",
        debug=False,
        enable_asserts=False,
        target_bir_lowering=False,
        num_devices=NCORES,
    )

    x_d = nc.dram_tensor("x", [N, D], F32, kind="ExternalInput").ap()
    w1_d = nc.dram_tensor("W1", [H, D], F32, kind="ExternalInput").ap()
    b1_d = nc.dram_tensor("b1r", [1, H], F32, kind="ExternalInput").ap()
    w2_d = nc.dram_tensor("W2", [O, H], F32, kind="ExternalInput").ap()
    idx_d = nc.dram_tensor("idx", [PP, 4], I32, kind="ExternalInput").ap()
    shard_d = nc.dram_tensor(
        "shard", [128 // NCORES, HC, VW], F32, kind="ExternalOutput"
    ).ap()

    with tile.TileContext(nc) as tc:
        _body(tc, x_d, w1_d, b1_d, w2_d, idx_d, shard_d)
    nc.compile()
    return nc


def _body(tc, x_d, w1_d, b1_d, w2_d, idx_d, shard_d):
    nc = tc.nc
    from contextlib import ExitStack

    ctx = ExitStack()
    singles = ctx.enter_context(tc.tile_pool(name="singles", bufs=1))
    work = ctx.enter_context(tc.tile_pool(name="work", bufs=1))
    ps_z = ctx.enter_context(tc.tile_pool(name="ps_z", bufs=2, space="PSUM"))
    ps_t = ctx.enter_context(tc.tile_pool(name="ps_t", bufs=2, space="PSUM"))
    ps_w = ctx.enter_context(tc.tile_pool(name="ps_w", bufs=4, space="PSUM"))
    dram = ctx.enter_context(tc.tile_pool(name="dram", bufs=1, space="DRAM"))

    # ---- gathers first: idx load gates them, they gate everything ----
    idx_sb = singles.tile([PP, 4], I32)
    nc.sync.dma_start(out=idx_sb[:], in_=idx_d[:])
    xg_t = [work.tile([128, D], F32, name=f"xg{j}") for j in range(4)]
    for j in range(4):
        nc.gpsimd.indirect_dma_start(
            out=xg_t[j][:],
            out_offset=None,
            in_=x_d[:],
            in_offset=bass.IndirectOffsetOnAxis(ap=idx_sb[:, j : j + 1], axis=0),
        )
    xg = [t[:] for t in xg_t]

    # ---- weight/bias loads overlap the gathers ----
    w1_sb = singles.tile([128, HC, D], F32)
    for hc in range(HC):
        nc.sync.dma_start(
            out=w1_sb[:, hc, :], in_=w1_d[hc * 128 : (hc + 1) * 128, :]
        )
    b1row = singles.tile([1, H], F32)
    nc.sync.dma_start(out=b1row[:], in_=b1_d[:])
    b1row_r = singles.tile([1, H], F32R)
    nc.vector.tensor_copy(out=b1row_r[:], in_=b1row[:])
    w2_sb = singles.tile([O, H], F32)
    nc.sync.dma_start(out=w2_sb[:], in_=w2_d[:])

    ident = singles.tile([128, 128], F32)
    make_identity(nc, ident[:])
    ones_rf = singles.tile([1, 128], F32)
    nc.vector.memset(ones_rf[:], 1.0)
    ones_r = singles.tile([1, 128], F32R)
    nc.vector.tensor_copy(out=ones_r[:], in_=ones_rf[:])
    ones64 = singles.tile([O, 1], F32)
    nc.vector.memset(ones64[:], 1.0)

    # ---- c = colsum(W2^2): early PE work (also warms the PE p-state) ----
    w2sq = singles.tile([O, H], F32)
    nc.vector.tensor_mul(w2sq[:], w2_sb[:], w2_sb[:])
    c_sb = singles.tile([128, HC], F32)
    for hc in range(HC):
        cp = ps_w.tile([128, 1], F32, tag="wp", name="cp")
        nc.tensor.matmul(
            cp[:], lhsT=w2sq[:, hc * 128 : (hc + 1) * 128], rhs=ones64[:],
            start=True, stop=True,
        )
        nc.scalar.copy(out=c_sb[:, hc : hc + 1], in_=cp[:])

    # ---- transposes: x (j0,j1) first, W1 next, x (j2,j3) as gathers land ----
    w1t = [singles.tile([128, H], F32R, name=f"w1t{dc}") for dc in range(DC)]
    xgt = [work.tile([128, DC, 128], F32R, name=f"xgt{j}") for j in range(4)]
    ha = [work.tile([128, H], BF16, name=f"ha{j}") for j in range(4)]
    copy_eng = [nc.vector.tensor_copy, nc.scalar.copy]

    def emit_xt(j):
        st = ps_t.tile([128, DC * 128], F32, tag="tp", name=f"stx{j}")
        for dc in range(DC):
            nc.tensor.transpose(
                st[:, dc * 128 : (dc + 1) * 128],
                xg[j][:, dc * 128 : (dc + 1) * 128], ident[:]
            )
        copy_eng[j % 2](out=xgt[j][:, :, :].opt(), in_=st[:])

    def emit_z(j):
        zp = ps_z.tile([128, H], F32, tag="z")
        nc.tensor.matmul(
            zp[:], lhsT=ones_r[:], rhs=b1row_r[:], start=True, stop=False,
        )
        for dc in range(DC):
            nc.tensor.matmul(
                zp[:], lhsT=xgt[j][:, dc, :], rhs=w1t[dc][:],
                start=False, stop=(dc == DC - 1),
            )
        nc.scalar.activation(out=ha[j][:], in_=zp[:], func=AF.Tanh)

    emit_xt(0)
    emit_xt(1)
    for dc in range(DC):
        st = ps_t.tile([128, H], F32, tag="tp", name=f"stw{dc}")
        for hc in range(HC):
            nc.tensor.transpose(
                st[:, hc * 128 : (hc + 1) * 128],
                w1_sb[:, hc, dc * 128 : (dc + 1) * 128], ident[:]
            )
        nc.scalar.copy(out=w1t[dc][:], in_=st[:])
    emit_z(0)
    emit_z(1)
    emit_xt(2)
    emit_xt(3)
    emit_z(2)
    emit_z(3)

    # ---- per-block V and U tiles (bf16) ----
    # k-order per block: U [da^2, db^2, dadb, hd] <-> V [vaa, vbb, vab, vhd]
    # (latest-ready tiles last); the -2 of the cross term lives on V.
    u_tiles = []
    v_tiles = []
    for blk in range(2):
        sgn = 1.0 if blk == 0 else -1.0
        xa, xb = xg[2 * blk], xg[2 * blk + 1]
        vaa = work.tile([128, VW], BF16, name=f"vaa{blk}")
        vab = work.tile([128, VW], BF16, name=f"vab{blk}")
        vbb = work.tile([128, VW], BF16, name=f"vbb{blk}")
        if blk == 0:
            nc.scalar.square(out=vaa[:, :D], in_=xa[:])
            nc.scalar.square(out=vbb[:, :D], in_=xb[:])
            t2 = work.tile([128, D], BF16, name="t2p")
            nc.vector.tensor_mul(t2[:], xa[:], xb[:])
            nc.vector.tensor_scalar_mul(vab[:, :D], t2[:], -2.0)
        else:
            nxa = work.tile([128, D], F32, name="nxa")
            nxb = work.tile([128, D], F32, name="nxb")
            nc.vector.tensor_scalar_mul(nxa[:], xa[:], -1.0)
            nc.vector.tensor_scalar_mul(nxb[:], xb[:], -1.0)
            nc.vector.tensor_mul(vaa[:, :D], xa[:], nxa[:])
            nc.vector.tensor_mul(vbb[:, :D], xb[:], nxb[:])
            t2 = work.tile([128, D], BF16, name="t2n")
            nc.vector.tensor_mul(t2[:], nxa[:], xb[:])   # -xa*xb
            nc.vector.tensor_scalar_mul(vab[:, :D], t2[:], -2.0)  # +2*xa*xb
        # constant columns (DVE memsets keep Pool clear for gather descgen):
        # col D = b1d weight {+1,-2,+1}*sgn, col D+1 = 0 (hd one-hot on vhd)
        nc.vector.memset(vaa[:, D : D + 1], sgn)
        nc.vector.memset(vab[:, D : D + 1], -2.0 * sgn)
        nc.vector.memset(vbb[:, D : D + 1], sgn)
        for v in (vaa, vab, vbb):
            nc.vector.memset(v[:, D + 1 : D + 2], 0.0)
        vhd = work.tile([128, VW], BF16, name=f"vhd{blk}")
        nc.vector.memset(vhd[:], 0.0)
        nc.vector.memset(vhd[:, D + 1 : D + 2], sgn)
        v_tiles += [vaa, vbb, vab, vhd]

        a, b = ha[2 * blk], ha[2 * blk + 1]
        ha_sq = work.tile([128, H], BF16, name=f"hasq{blk}")
        hb_sq = work.tile([128, H], BF16, name=f"hbsq{blk}")
        nc.vector.tensor_mul(ha_sq[:], a[:], a[:])
        nc.vector.tensor_mul(hb_sq[:], b[:], b[:])
        da_sq = work.tile([128, H], BF16, name=f"dasq{blk}")
        db_sq = work.tile([128, H], BF16, name=f"dbsq{blk}")
        # (1 - h^2)^2 in one ACT op: Square(-x + 1) applied to h^2
        nc.scalar.activation(out=da_sq[:], in_=ha_sq[:], func=AF.Square,
                             bias=1.0, scale=-1.0)
        nc.scalar.activation(out=db_sq[:], in_=hb_sq[:], func=AF.Square,
                             bias=1.0, scale=-1.0)
        da = work.tile([128, H], BF16, name=f"da{blk}")
        db = work.tile([128, H], BF16, name=f"db{blk}")
        nc.vector.tensor_scalar(da[:], ha_sq[:], -1.0, 1.0, ALU.mult, ALU.add)
        nc.vector.tensor_scalar(db[:], hb_sq[:], -1.0, 1.0, ALU.mult, ALU.add)
        dadb = work.tile([128, H], BF16, name=f"dadb{blk}")
        nc.vector.tensor_mul(dadb[:], da[:], db[:])
        hd_d = work.tile([128, H], BF16, name=f"hdd{blk}")
        hd = work.tile([128, H], BF16, name=f"hd{blk}")
        nc.gpsimd.tensor_sub(hd_d[:], a[:], b[:])
        nc.scalar.square(out=hd[:], in_=hd_d[:])
        u_tiles += [da_sq, db_sq, dadb, hd]

    # ---- big matmul (bf16) + c post-scale -> partial [128, HC, VW] ----
    # pos half runs while the neg block is still in prep
    partial = work.tile([128, HC, VW], F32)
    cc_in = dram.tile([128, HC, VW], F32)
    wps = [ps_w.tile([128, VW], F32, tag="wp", name=f"wp{hc}") for hc in range(HC)]
    for half in range(2):
        for hc in range(HC):
            wp = wps[hc]
            for k in range(4):
                kk = 4 * half + k
                nc.tensor.matmul(
                    wp[:], lhsT=u_tiles[kk][:, hc * 128 : (hc + 1) * 128],
                    rhs=v_tiles[kk][:], start=(kk == 0), stop=(kk == 7),
                )
            if half == 0:
                continue
            # W1d cols + b1d col scale by c; hd col copied raw
            if hc % 2 == 0:
                nc.vector.tensor_scalar_mul(
                    partial[:, hc, : D + 1], wp[:, : D + 1], c_sb[:, hc : hc + 1]
                )
                nc.scalar.copy(out=partial[:, hc, D + 1 : VW], in_=wp[:, D + 1 : VW])
            else:
                nc.scalar.activation(
                    out=partial[:, hc, : D + 1], in_=wp[:, : D + 1],
                    func=AF.Copy, scale=c_sb[:, hc : hc + 1],
                )
                nc.vector.tensor_copy(out=partial[:, hc, D + 1 : VW], in_=wp[:, D + 1 : VW])

    nc.sync.dma_start(out=cc_in[:], in_=partial[:])

    # ---- ReduceScatter over the 8 cores (collectives can't write IO) ----
    SH = 128 // NCORES
    rs_out = dram.tile([SH, HC, VW], F32)
    nc.gpsimd.collective_compute(
        "ReduceScatter",
        ALU.add,
        replica_groups=[list(range(NCORES))],
        ins=[cc_in.opt()],
        outs=[rs_out.opt()],
    )
    nc.sync.dma_start(out=shard_d[:], in_=rs_out[:])
    ctx.close()


def _get_program():
    if "nc" not in _CACHE:
        _CACHE["nc"] = _build_program()
    return _CACHE["nc"]


def kernel(**inputs):
    x = np.ascontiguousarray(np.asarray(inputs["x"], dtype=np.float32))
    W1 = np.ascontiguousarray(np.asarray(inputs["W1"], dtype=np.float32))
    b1 = np.ascontiguousarray(
        np.asarray(inputs["b1"], dtype=np.float32).reshape(1, H)
    )
    W2 = np.ascontiguousarray(np.asarray(inputs["W2"], dtype=np.float32))
    iap = np.asarray(inputs["ap"], dtype=np.int32)
    ip = np.asarray(inputs["p"], dtype=np.int32)
    ian = np.asarray(inputs["an"], dtype=np.int32)
    inn = np.asarray(inputs["n"], dtype=np.int32)

    nc = _get_program()
    in_maps = []
    for i in range(NCORES):
        s = slice(i * PP, (i + 1) * PP)
        idx = np.ascontiguousarray(
            np.stack([iap[s], ip[s], ian[s], inn[s]], axis=1).astype(np.int32)
        )
        in_maps.append({"x": x, "W1": W1, "b1r": b1, "W2": W2, "idx": idx})

    res = bass_utils.run_bass_kernel_spmd(
        nc, in_maps, core_ids=list(range(NCORES))
    )
    return _assemble([res.results[c] for c in range(NCORES)])


def _assemble(per_core):
    """Pure gather/unshard: concatenate the ReduceScatter shards and the
    device-computed W2d/b2d tail into the full [164416] output."""
    shards = np.stack([per_core[c]["shard"] for c in range(NCORES)])  # [8,16,HC,VW]
    red = shards.transpose(2, 0, 1, 3).reshape(H, VW)  # h = hc*128 + 16c + q
    out = np.empty(NPARAM, np.float32)
    out[0 : H * D] = red[:, :D].reshape(-1)
    out[H * D : H * D + H] = red[:, D]
    base = H * D + H
    out[base : base + O * H] = np.tile(red[:, D + 1], O)  # W2d rows all equal hd
    out[base + O * H :] = 0.0  # b2d is exactly zero
    return out
